# revision 48
# baseline (speedup 1.0000x reference)
"""TRN2 Bass kernel for nn_Encoder_27290222198965.

Reference computation (N=8, L=2048, H=1024):
    q = x@Wq.T+bq ; k = x@Wk.T+bk ; v = x@Wv.T+bv
    d[n,l] = sum_h q*k                       (diagonal "attention" scores)
    att = softmax(diag-embed(d), axis=2) ->  colsum[n,l] = S[n] + (e-1)/(L-1+e),
        e = exp(d[n,l]), S[n] = sum_l 1/(L-1+exp(d[n,l]))
    out = (colsum[:, :, None] * v) @ Wo.T + bo

Algebraic refactor (validated to ~4e-6 rel err with exact matmuls):
    d[n,l] = rowsum(x ⊙ y') + c0,  y' = x @ M^T + u,
        M = Wq^T Wk, u = Wk^T bq + Wq^T bk, c0 = bq·bk
    colsum = (S+1) - 2048*r,  r = 1/(2047+exp(d)),  S = sum_l r
        (uses e*r = 1 - 2047*r)
    out    = colsum ⊙ (x @ Wc^T + bc) + bo,  Wc = Wo@Wv, bc = Wo@bv
so only TWO HxH projections run on hardware (y' and z) instead of four.

Sharding: data-parallel over N — core n handles batch n. All matmuls in
float32r (full PE rate at free dim 512, ~e8m12 effective precision).
Everything on-chip is transposed ([feature, token]) so biases are
per-partition and fold into ScalarE psum->sbuf copies.

x is loaded ONCE into four resident SBUF tiles [128, 8hb, 512] (one per
l-block, 8 per-hb chunk DMAs each) and reused by phase 2 — this removes
the per-matmul DMA-completion semaphore waits that inflated the PE
stream from 227ns/MM to 272ns/MM, and cuts HBM traffic 32MB -> 24MB.
"""

import numpy as np

import concourse.bass as bass  # noqa: F401  (registers engines on Bacc)
import concourse.tile as tile
from concourse import bacc, mybir
from concourse.bass_utils import run_bass_kernel_spmd

dt = mybir.dt
AF = mybir.ActivationFunctionType
ALU = mybir.AluOpType

N, L, H = 8, 2048, 1024
P = 128            # SBUF partitions
LB = 512           # l-block (moving free dim of every matmul)
NH = H // P        # 8 h-blocks
NL = L // LB       # 4 l-blocks
N_CORES = 8

_CACHE = {}


def _build():
    nc = bacc.Bacc("TRN2", target_bir_lowering=False, debug=False,
                   num_devices=N_CORES)

    CPW = NH + 1 + P + 2 * NH   # ub | c0 | ones | bcb | bob
    xT_d = nc.dram_tensor("xT", [H, L], dt.bfloat16, kind="ExternalInput").ap()
    MT_d = nc.dram_tensor("MT", [NH, P, NH * P], dt.bfloat16, kind="ExternalInput").ap()
    WcT_d = nc.dram_tensor("WcT", [NH, P, NH * P], dt.bfloat16, kind="ExternalInput").ap()
    cp_d = nc.dram_tensor("cpack", [P, CPW], dt.float32r,
                          kind="ExternalInput").ap()
    out_d = nc.dram_tensor("outT", [H, L], dt.float32, kind="ExternalOutput").ap()

    xT3 = xT_d.rearrange("(j p) l -> p j l", p=P)    # [128, 8, 2048]
    MT3 = MT_d    # prepacked [ob, p(hin%128), hb*128+hout]
    WcT3 = WcT_d

    with tile.TileContext(nc) as tc:
        with (
            tc.tile_pool(name="resident", bufs=1) as rp,
            tc.tile_pool(name="weights", bufs=1) as wtp,
            tc.tile_pool(name="work", bufs=3) as wp,
            tc.tile_pool(name="obuf", bufs=2) as op2,
            tc.tile_pool(name="mmpsum", bufs=6, space="PSUM") as yp,
            tc.tile_pool(name="dpsum", bufs=2, space="PSUM") as dp,
        ):
            t_s = rp.tile([P, L], dt.float32)
            cs = rp.tile([P, L], dt.float32)

            def load_w(src3, ob, tag, split_first=False):
                """One per-ob weight tile [hin(P), hb*P+hout] = 256KB."""
                t = wtp.tile([P, NH * P], dt.bfloat16, tag=f"{tag}{ob}")
                if split_first:
                    # land the first 32KB stripe early so the first
                    # LDWEIGHTS doesn't wait on the full 256KB
                    nc.scalar.dma_start(t[:, :P], src3[ob, :, :P])
                    nc.scalar.dma_start(t[:, P:], src3[ob, :, P:])
                else:
                    nc.scalar.dma_start(t[:], src3[ob])
                return t

            # resident x: one tile per l-block, [128, hb, 512]; chunked DMA
            # per hb so the first matmuls don't wait on the full 2MB.
            xl = [rp.tile([P, NH, LB], dt.bfloat16, name=f"xl{lb}", tag=f"xl{lb}")
                  for lb in range(NL)]

            def load_x(lb, chunked=False):
                src = xT3[:, :, lb * LB:(lb + 1) * LB]
                if not chunked:      # one 1MB DMA (fewer semaphores)
                    nc.sync.dma_start(xl[lb][:], src)
                    return
                # geometric chunk ladder: early matmuls start on a 64KB
                # transfer while later hb blocks stream in bigger pieces
                # (each DMA_DIRECT2D costs ~600ns of queue-engine issue
                # time, so few-but-growing chunks beat per-hb chunks)
                nc.sync.dma_start(xl[lb][:, 0, :LB // 2], src[:, 0, :LB // 2])
                nc.sync.dma_start(xl[lb][:, 0, LB // 2:], src[:, 0, LB // 2:])
                nc.sync.dma_start(xl[lb][:, 1, :], src[:, 1])
                nc.sync.dma_start(xl[lb][:, 2:4, :], src[:, 2:4])
                nc.sync.dma_start(xl[lb][:, 4:, :], src[:, 4:])

            # ---- JIT DMA emission for the cold start ----
            cp = rp.tile([P, CPW], dt.float32r)
            mt = [None] * NH
            mt[0] = load_w(MT3, 0, "mt", split_first=True)
            load_x(0, chunked=True)
            nc.sync.dma_start(cp[:], cp_d[:])
            ub = cp[:, :NH].bitcast(dt.float32)
            c0b = cp[:, NH:NH + 1].bitcast(dt.float32)
            ones = cp[:, NH + 1:NH + 1 + P]
            bcb = cp[:, NH + 1 + P:NH + 1 + P + NH].bitcast(dt.float32)
            bob = cp[:, NH + 1 + P + NH:].bitcast(dt.float32)
            for ob in range(1, 3):
                mt[ob] = load_w(MT3, ob, "mt")
            # mt[3:] issued later (inside lb0) so the x stream gets more
            # DMA bandwidth during the cold start

            # d-matmul bookkeeping: delay each block's last rowsum-MM into the
            # next MM group so the PE never waits on the ACT->DVE prod chain.
            state = {"pending": None}

            def flush_pending():
                if state["pending"] is None:
                    return
                pd_t, ob, prod_t, is_last, lb = state["pending"]
                nc.tensor.matmul(pd_t[:], ones, prod_t[:],
                                 start=(ob == 0), stop=is_last)
                state["pending"] = None
                if is_last:
                    # t = sigmoid(-d - c0 + ln(L-1)); r = t/(L-1)
                    # (1/((L-1)+e^d) = sigmoid(-d+ln(L-1))/(L-1))
                    ls = slice(lb * LB, (lb + 1) * LB)
                    nc.scalar.activation(t_s[:, ls], pd_t[:], AF.Sigmoid,
                                         bias=c0b[:, 0:1], scale=-1.0)

            # ================= phase 1: y' ; d ; r ==================
            for lb in range(NL):
                pd = dp.tile([P, LB], dt.float32)
                acc = None
                for ob in range(NH):
                    # first two groups run as 2x256-wide chunks so the very
                    # first matmul waits on only 128KB of x + 64KB of weights
                    nmm = 2 if (lb == 0 and ob < 2) else 1
                    mw = LB // nmm
                    pys = []
                    for ck in range(nmm):
                        py = yp.tile([P, mw], dt.float32, tag="mm")
                        for hb in range(NH):
                            nc.tensor.matmul(
                                py[:], mt[ob][:, hb * P:(hb + 1) * P],
                                xl[lb][:, hb, ck * mw:(ck + 1) * mw],
                                start=(hb == 0), stop=(hb == NH - 1))
                        pys.append(py)
                    if ob == 1:
                        flush_pending()
                    if lb == 0 and ob == 0:
                        for o2 in range(3, NH):
                            mt[o2] = load_w(MT3, o2, "mt")
                    if lb == 0 and ob == 2:
                        # deferred so the weight stream gets full DMA
                        # bandwidth during the first groups
                        load_x(1)
                    yb = wp.tile([P, LB], dt.float32, tag="yb")
                    for ck in range(nmm):
                        nc.scalar.activation(
                            yb[:, ck * mw:(ck + 1) * mw], pys[ck][:],
                            AF.Identity, bias=ub[:, ob:ob + 1], scale=1.0)
                    prod = wp.tile([P, LB], dt.float32r, tag="prod")
                    nc.vector.tensor_tensor(
                        prod[:], yb[:], xl[lb][:, ob, :], op=ALU.mult)
                    if acc is None:
                        acc = prod
                    else:
                        nacc = wp.tile([P, LB], dt.float32r, tag="pacc")
                        nc.vector.tensor_tensor(nacc[:], acc[:], prod[:],
                                                op=ALU.add)
                        acc = nacc
                state["pending"] = (pd, 0, acc, True, lb)
                # prefetch x for block lb+2 of phase 1, then phase-2 weights
                nxt = lb + 2
                if nxt < NL:
                    load_x(nxt)
                elif nxt == NL:  # after block 2: phase-2 weights
                    wct = [load_w(WcT3, ob, "wct") for ob in range(NH)]

            # ================= phase 2: z ; out ==================
            # ob-outer: each ob's four l-blocks drain consecutively into one
            # [128, 2048] buffer stored with a single 1MB DMA (ob<7) — far
            # fewer DMAs/semaphores; the last ob streams per-l-block stores.
            for ob in range(NH):
                obuf = op2.tile([P, L], dt.float32, tag="obuf")
                orow = out_d[ob * P:(ob + 1) * P, :]
                for lb in range(NL):
                    last_grp = (ob == NH - 1 and lb == NL - 1)
                    nmm = 4 if last_grp else 1   # bf16: full rate at 128 wide
                    mw = LB // nmm
                    pzs = []
                    for ck in range(nmm):
                        pz = yp.tile([P, mw], dt.float32, tag="mm")
                        for hb in range(NH):
                            nc.tensor.matmul(
                                pz[:], wct[ob][:, hb * P:(hb + 1) * P],
                                xl[lb][:, hb, ck * mw:(ck + 1) * mw],
                                start=(hb == 0), stop=(hb == NH - 1))
                        pzs.append(pz)
                    if ob == 0 and lb == 0:
                        flush_pending()   # last d-MM of phase 1
                        # colsum = (1 + sum(t)/(L-1)) - (L/(L-1))*t
                        S_t = rp.tile([P, 1], dt.float32)
                        nc.vector.tensor_reduce(
                            S_t[:], t_s[:], axis=mybir.AxisListType.X,
                            op=ALU.add)
                        S1_t = rp.tile([P, 1], dt.float32)
                        nc.vector.tensor_scalar(
                            S1_t[:], S_t[:], 1.0 / (L - 1), 1.0,
                            op0=ALU.mult, op1=ALU.add)
                        nc.vector.tensor_scalar(
                            cs[:], t_s[:], -float(L) / (L - 1), S1_t[:],
                            op0=ALU.mult, op1=ALU.add)
                    for ck in range(nmm):
                        lo = lb * LB + ck * mw
                        lsc = slice(lo, lo + mw)
                        zb = wp.tile([P, mw], dt.float32, tag="zb")
                        nc.scalar.activation(zb[:], pzs[ck][:], AF.Identity,
                                             bias=bcb[:, ob:ob + 1], scale=1.0)
                        zc = wp.tile([P, mw], dt.float32, tag="zc")
                        nc.vector.tensor_tensor(zc[:], zb[:], cs[:, lsc],
                                                op=ALU.mult)
                        nc.vector.tensor_scalar_add(
                            obuf[:, lsc], zc[:], bob[:, ob:ob + 1])
                        if ob == NH - 1:
                            # last ob: stream stores as drains complete,
                            # alternating issue queues (sync/scalar) so the
                            # ~600ns-per-DMA issue cost doesn't serialize
                            # into the kernel tail
                            if lb == 0:
                                pass          # combined with lb1 below
                            elif lb == 1:
                                nc.scalar.dma_start(orow[:, :2 * LB],
                                                    obuf[:, :2 * LB])
                            elif lb == 2:
                                nc.sync.dma_start(orow[:, lsc], obuf[:, lsc])
                            else:
                                q = nc.scalar if ck % 2 == 0 else nc.sync
                                q.dma_start(orow[:, lsc], obuf[:, lsc])
                if ob < NH - 1:
                    nc.sync.dma_start(orow, obuf[:])

    nc.compile()
    return nc


def _get_nc():
    if "nc" not in _CACHE:
        _CACHE["nc"] = _build()
    return _CACHE["nc"]


def _prep_inputs(x, Wq, bq, Wk, bk, Wv, bv, Wo, bo):
    """Host-side precompute (fp64 for the fused weights) + per-core sharding."""
    f8 = np.float64
    M = (Wq.astype(f8).T @ Wk.astype(f8)).astype(np.float32)
    u = (Wk.astype(f8).T @ bq.astype(f8)
         + Wq.astype(f8).T @ bk.astype(f8)).astype(np.float32)
    c0 = np.float32(bq.astype(f8) @ bk.astype(f8))
    Wc = (Wo.astype(f8) @ Wv.astype(f8)).astype(np.float32)
    bc = (Wo.astype(f8) @ bv.astype(f8)).astype(np.float32)

    import ml_dtypes

    def _pack(WT):  # [H,H] (hin, hout) -> [NH(ob), P(hin%P), NH*P], bf16
        t = WT.reshape(NH, P, NH, P)          # [hb, p, ob, c]
        return np.ascontiguousarray(
            t.transpose(2, 1, 0, 3).reshape(NH, P, NH * P)
        ).astype(ml_dtypes.bfloat16)

    MT = _pack(M.T)
    WcT = _pack(Wc.T)
    ub = np.ascontiguousarray(u.reshape(NH, P).T)
    bcb = np.ascontiguousarray(bc.reshape(NH, P).T)
    bob = np.ascontiguousarray(bo.astype(np.float32).reshape(NH, P).T)
    c0b = np.full((P, 1), np.log(L - 1.0) - np.float64(c0), np.float32)
    ones = np.ones((P, P), np.float32)
    cpack = np.concatenate([ub, c0b, ones, bcb, bob], axis=1)

    shared = dict(MT=MT, WcT=WcT, cpack=cpack)
    in_maps = []
    for n in range(N_CORES):
        xT = np.ascontiguousarray(x[n].astype(np.float32).T).astype(
            ml_dtypes.bfloat16)
        in_maps.append(dict(xT=xT, **shared))
    return in_maps


def kernel(x, Wq, bq, Wk, bk, Wv, bv, Wo, bo, _trace=False, _trace_kwargs=None):
    x, Wq, bq, Wk, bk, Wv, bv, Wo, bo = (
        np.asarray(a) for a in (x, Wq, bq, Wk, bk, Wv, bv, Wo, bo))
    nc = _get_nc()
    in_maps = _prep_inputs(x, Wq, bq, Wk, bk, Wv, bv, Wo, bo)
    res = run_bass_kernel_spmd(nc, in_maps, list(range(N_CORES)),
                               trace=_trace, **(_trace_kwargs or {}))
    out = np.empty((N, L, H), np.float32)
    for n in range(N_CORES):
        out[n] = res.results[n]["outT"].T
    if _trace:
        kernel.last_result = res
    return out
